# revision 1
# baseline (speedup 1.0000x reference)
"""Multi-head attention (B=2, S=2048, D=1024, H=16) on 8 Trainium2 cores.

Sharding: core c -> (batch b = c//4, head-group hg = c%4 of 4 heads, d_h=256).
Megatron-style: column-shard W_{q,k,v}, row-shard W_o; partial outputs are
summed on the host (the unshard step).

Per-core pipeline (activations kept transposed, "T-space", fp32r matmuls):
  kT = (Wk_hg/8) @ K_b^T        [256, 2048]
  v  = V_b @ Wv_hg^T (+ones col)[2048, 4*65]
  per 512-wide q-chunk qc:
    qT[:, qc] = Wq_hg @ Q_b^T[:, qc]
    per head pair, per kt pair: sT[kpos, q] = kT_h-slices^T @ qT_h (row-packed)
    pT = exp(sT)            (ScalarE, 1024-wide over 2 PSUM banks)
    ctxT (+denom row) = v_h_aug^T @ pT   (M=65, accumulated over 16 kt)
    ctxT /= denom           (DVE recip + DRAM-roundtrip partition broadcast)
    oT_partial[:, qc] = Wo_cols^T-slices @ ctxT
Host: out[b] = (sum over the 4 cores of batch b of oT).T + bo.
"""

import numpy as np

import bass_rust
import concourse.bass as bass
import concourse.mybir as mybir
import concourse.tile as tile
from concourse.bass_utils import run_bass_kernel_spmd

F32 = mybir.dt.float32
F32R = mybir.dt.float32r

B, S, D = 2, 2048, 1024
H = 16
DK = 64
N_CORES = 8
HEADS_PER_CORE = 4          # d_h = 256
DH = HEADS_PER_CORE * DK    # 256
VW = DK + 1                 # v columns per head incl. ones column
NV = HEADS_PER_CORE * VW    # 260
QC = 512                    # q-chunk (PSUM bank = 512 fp32)
N_QC = S // QC              # 4
N_KT = S // 128             # 16 key tiles
N_KO = D // 128             # 8 contraction tiles for projections
MT = DH // 128              # 2 m-tiles for qT/kT/ctxT


def _legalize_waits(nc):
    """walrus here allows 1 sync-wait per instruction (2 for EventSemaphore);
    Tile emits more. Spill extras onto same-engine NoOps placed just before."""
    caps = {"InstEventSemaphore": 2}
    n_nops = 0
    for f in nc.m.functions:
        for bb in f.blocks:
            insts = bb.instructions
            out = []
            changed = False
            for inst in insts:
                si = inst.sync_info
                waits = list(si.on_wait) if si is not None else []
                cap = caps.get(type(inst).__name__, 1)
                if len(waits) > cap:
                    spill, keep = waits[:-cap], waits[-cap:]
                    for w in spill:
                        nop = mybir.InstNoOp(name=f"waitfix-{n_nops}", ins=[], outs=[])
                        n_nops += 1
                        nop.engine = inst.engine
                        nop.sync_info = bass_rust.SyncInfo(on_wait=[w], on_update=[])
                        out.append(nop)
                    si.on_wait = keep
                    changed = True
                out.append(inst)
            if changed:
                insts[:] = out
    return n_nops


def build_nc(reps: int = 1):
    nc = bass.Bass(num_devices=N_CORES)

    t = {}
    t["xqT"] = nc.dram_tensor("xqT", [D, S], F32, kind="ExternalInput")
    t["xkT"] = nc.dram_tensor("xkT", [D, S], F32, kind="ExternalInput")
    t["xvT"] = nc.dram_tensor("xvT", [D, S], F32, kind="ExternalInput")
    t["wqT"] = nc.dram_tensor("wqT", [D, DH], F32, kind="ExternalInput")
    t["wkT"] = nc.dram_tensor("wkT", [D, DH], F32, kind="ExternalInput")
    t["wvT"] = nc.dram_tensor("wvT", [D, NV], F32, kind="ExternalInput")
    t["bq"] = nc.dram_tensor("bq", [DH], F32, kind="ExternalInput")
    t["bk"] = nc.dram_tensor("bk", [DH], F32, kind="ExternalInput")
    t["bv_bc"] = nc.dram_tensor("bv_bc", [128, NV], F32, kind="ExternalInput")
    t["woT"] = nc.dram_tensor("woT", [DH, D], F32, kind="ExternalInput")
    t["oT"] = nc.dram_tensor("oT", [D, S], F32, kind="ExternalOutput")

    with tile.TileContext(nc) as tc:
        _body(nc, tc, t, reps)
    _legalize_waits(nc)
    return nc


def _body(nc, tc, t, reps):
    from contextlib import ExitStack

    with ExitStack() as ctx:
        singles = ctx.enter_context(tc.tile_pool(name="singles", bufs=1))

        wq_s = singles.tile([128, N_KO, DH], F32R)
        wk_s = singles.tile([128, N_KO, DH], F32R)
        wv_s = singles.tile([128, N_KO, NV], F32R)
        wo_s = singles.tile([128, MT, D], F32R)
        bq_s = singles.tile([128, MT], F32)
        bk_s = singles.tile([128, MT], F32)
        bv_s = singles.tile([128, NV], F32)
        nc.sync.dma_start(wq_s[:], t["wqT"].rearrange("(ko p) m -> p ko m", p=128).bitcast(F32R))
        nc.sync.dma_start(wk_s[:], t["wkT"].rearrange("(ko p) m -> p ko m", p=128).bitcast(F32R))
        nc.sync.dma_start(wv_s[:], t["wvT"].rearrange("(ko p) m -> p ko m", p=128).bitcast(F32R))
        nc.sync.dma_start(wo_s[:], t["woT"].rearrange("(kt p) e -> p kt e", p=128).bitcast(F32R))
        nc.sync.dma_start(bq_s[:], t["bq"].rearrange("(m p) -> p m", p=128))
        nc.sync.dma_start(bk_s[:], t["bk"].rearrange("(m p) -> p m", p=128))
        nc.sync.dma_start(bv_s[:], t["bv_bc"][:, :])

        qT_s = singles.tile([128, MT, S], F32R)
        kT_s = singles.tile([128, MT, S], F32R)
        v_s = singles.tile([128, N_KT, NV], F32R)
        ctxT_s = singles.tile([128, MT, S], F32R)

        for _ in range(reps):
            with ExitStack() as ictx:
                _compute(nc, tc, ictx, t, wq_s, wk_s, wv_s, bq_s, bk_s, bv_s,
                         wo_s, qT_s, kT_s, v_s, ctxT_s)


def _proj_qk(nc, xin, proj_ps, xdram, w_s, b_s, dst, sc):
    """Project one 512-col chunk of qT or kT: dst[:, :, sc*QC:+QC]."""
    xt = xin.tile([128, N_KO, QC], F32R, tag="x", name="xt")
    nc.sync.dma_start(
        xt[:],
        xdram.rearrange("(ko p) s -> p ko s", p=128)[
            :, :, sc * QC:(sc + 1) * QC
        ].bitcast(F32R),
    )
    for m in range(MT):
        ps = proj_ps.tile([128, QC], F32, tag="proj", name="proj_ps")
        for ko in range(N_KO):
            nc.tensor.matmul(
                ps[:],
                w_s[:, ko, m * 128:(m + 1) * 128],
                xt[:, ko],
                start=(ko == 0),
                stop=(ko == N_KO - 1),
            )
        nc.vector.tensor_tensor(
            dst[:, m, sc * QC:(sc + 1) * QC],
            ps[:],
            b_s[:, m, None].to_broadcast((128, QC)),
            mybir.AluOpType.add,
        )


def _proj_v(nc, xin, proj_ps, xvT, wv_s, bv_s, v_s, sc):
    xt = xin.tile([128, N_KO, QC], F32R, tag="x", name="xt")
    nc.sync.dma_start(
        xt[:],
        xvT.rearrange("(ko p) s -> p ko s", p=128)[
            :, :, sc * QC:(sc + 1) * QC
        ].bitcast(F32R),
    )
    for rt in range(QC // 128):
        ps = proj_ps.tile([128, QC], F32, tag="proj", name="proj_ps")
        for ko in range(N_KO):
            nc.tensor.matmul(
                ps[:, :NV],
                xt[:, ko, rt * 128:(rt + 1) * 128],
                wv_s[:, ko],
                start=(ko == 0),
                stop=(ko == N_KO - 1),
            )
        nc.vector.tensor_tensor(
            v_s[:, sc * (QC // 128) + rt, :],
            ps[:, :NV],
            bv_s[:],
            mybir.AluOpType.add,
        )


def _compute(nc, tc, ctx, t, wq_s, wk_s, wv_s, bq_s, bk_s, bv_s, wo_s,
             qT_s, kT_s, v_s, ctxT_s):
    # ---------- Stage A: projections ----------
    with tc.tile_pool(name="xin", bufs=3) as xin, \
         tc.tile_pool(name="proj_ps", bufs=4, space="PSUM") as proj_ps:
        for sc in range(N_QC):
            _proj_qk(nc, xin, proj_ps, t["xqT"], wq_s, bq_s, qT_s, sc)
        for sc in range(N_QC):
            _proj_qk(nc, xin, proj_ps, t["xkT"], wk_s, bk_s, kT_s, sc)
        for sc in range(N_QC):
            _proj_v(nc, xin, proj_ps, t["xvT"], wv_s, bv_s, v_s, sc)

    # ---------- Stage B: attention + output projection ----------
    with tc.tile_pool(name="pT", bufs=3) as pT_pool, \
         tc.tile_pool(name="norm", bufs=4) as norm_pool, \
         tc.tile_pool(name="ndram", bufs=4, space="DRAM") as ndram_pool, \
         tc.tile_pool(name="osb", bufs=2) as osb_pool, \
         tc.tile_pool(name="sc_ps", bufs=2, space="PSUM") as sc_ps, \
         tc.tile_pool(name="ctx_ps", bufs=1, space="PSUM") as ctx_ps, \
         tc.tile_pool(name="o_ps", bufs=2, space="PSUM") as o_ps:
        for qc in range(N_QC):
            for pair in range(HEADS_PER_CORE // 2):
                m = pair
                ctx_banks = [
                    ctx_ps.tile([128, QC], F32, tag=f"ctx{hl}", name=f"ctx{hl}")
                    for hl in range(2)
                ]
                # software pipeline: scores+exp for kt, PV consumes kt-1's
                # exp output so PE never waits on the ScalarE exp latency.
                pend = [None, None]  # per hl: (pt_tile, kt)
                for kt in range(N_KT):
                    for hl in range(2):
                        off = 64 * hl
                        ps_s = sc_ps.tile([128, QC], F32, tag=f"s{hl}", name=f"s{hl}")
                        nc.tensor.matmul(
                            ps_s[:],
                            kT_s[off:off + 64, m, kt * 128:(kt + 1) * 128],
                            qT_s[off:off + 64, m, qc * QC:(qc + 1) * QC],
                            start=True,
                            stop=True,
                        )
                        pt = pT_pool.tile([128, QC], F32R, tag=f"p{hl}", name=f"pt{hl}")
                        nc.scalar.activation(
                            pt[:], ps_s[:], mybir.ActivationFunctionType.Exp
                        )
                        prev = pend[hl]
                        pend[hl] = (pt, kt)
                        if prev is not None:
                            h = 2 * pair + hl
                            nc.tensor.matmul(
                                ctx_banks[hl][0:VW, :],
                                v_s[:, prev[1], h * VW:(h + 1) * VW],
                                prev[0][:],
                                start=(prev[1] == 0),
                                stop=False,
                            )
                for hl in range(2):
                    h = 2 * pair + hl
                    pt, kt = pend[hl]
                    nc.tensor.matmul(
                        ctx_banks[hl][0:VW, :],
                        v_s[:, kt, h * VW:(h + 1) * VW],
                        pt[:],
                        start=False,
                        stop=True,
                    )
                for hl in range(2):
                    r_s = norm_pool.tile([1, QC], F32, tag="r", name="r_s")
                    nc.vector.reciprocal(r_s[:], ctx_banks[hl][64:65, :])
                    r_d = ndram_pool.tile([1, QC], F32, tag="rd", name="r_d")
                    nc.gpsimd.dma_start(r_d[:], r_s[:])
                    rbc = norm_pool.tile([64, QC], F32, tag="rbc", name="rbc")
                    nc.gpsimd.dma_start(
                        rbc[:],
                        bass.AP(
                            tensor=r_d.tensor,
                            offset=r_d.offset,
                            ap=[[0, 64]] + list(r_d.ap[1:]),
                        ),
                    )
                    nc.vector.tensor_tensor(
                        ctxT_s[64 * hl:64 * hl + 64, m, qc * QC:(qc + 1) * QC],
                        ctx_banks[hl][0:64, :],
                        rbc[:],
                        mybir.AluOpType.mult,
                    )
            o_sb = osb_pool.tile([128, D // 128, QC], F32, tag="o", name="o_sb")
            for mt in range(D // 128):
                ps_o = o_ps.tile([128, QC], F32, tag="o", name="ps_o")
                for kt in range(MT):
                    nc.tensor.matmul(
                        ps_o[:],
                        wo_s[:, kt, mt * 128:(mt + 1) * 128],
                        ctxT_s[:, kt, qc * QC:(qc + 1) * QC],
                        start=(kt == 0),
                        stop=(kt == MT - 1),
                    )
                nc.vector.tensor_copy(o_sb[:, mt, :], ps_o[:])
            nc.sync.dma_start(
                t["oT"].rearrange("(mt p) s -> p mt s", p=128)[
                    :, :, qc * QC:(qc + 1) * QC
                ],
                o_sb[:],
            )


def shard_inputs(Q, K, V, Wq, bq, Wk, bk, Wv, bv, Wo, bo):
    """Host-side shard prep. Returns per-core in_maps."""
    scale = 1.0 / np.sqrt(np.float32(DK))
    in_maps = []
    xT = {}
    for b in range(B):
        xT[b] = (
            np.ascontiguousarray(np.asarray(Q[b]).T),
            np.ascontiguousarray(np.asarray(K[b]).T),
            np.ascontiguousarray(np.asarray(V[b]).T),
        )
    for c in range(N_CORES):
        b, hg = c // HEADS_PER_CORE, c % HEADS_PER_CORE
        rows = slice(DH * hg, DH * (hg + 1))
        wqT = np.ascontiguousarray(np.asarray(Wq)[rows].T)
        wkT = np.ascontiguousarray((np.asarray(Wk)[rows] * scale).T)
        wvT = np.zeros((D, NV), np.float32)
        bv_bc = np.zeros((128, NV), np.float32)
        for i in range(HEADS_PER_CORE):
            wr = slice(DH * hg + DK * i, DH * hg + DK * (i + 1))
            wvT[:, VW * i:VW * i + DK] = np.asarray(Wv)[wr].T
            bv_bc[:, VW * i:VW * i + DK] = np.asarray(bv)[wr][None, :]
            bv_bc[:, VW * i + DK] = 1.0
        woT = np.ascontiguousarray(np.asarray(Wo)[:, rows].T)
        in_maps.append(
            {
                "xqT": xT[b][0],
                "xkT": xT[b][1],
                "xvT": xT[b][2],
                "wqT": wqT,
                "wkT": wkT,
                "wvT": wvT,
                "bq": np.ascontiguousarray(np.asarray(bq)[rows]),
                "bk": np.ascontiguousarray(np.asarray(bk)[rows] * scale),
                "bv_bc": bv_bc,
                "woT": woT,
            }
        )
    return in_maps


def unshard(results, bo):
    out = np.empty((B, S, D), np.float32)
    for b in range(B):
        acc = results[b * HEADS_PER_CORE]["oT"].astype(np.float32).copy()
        for hg in range(1, HEADS_PER_CORE):
            acc += results[b * HEADS_PER_CORE + hg]["oT"]
        out[b] = acc.T + np.asarray(bo)[None, :]
    return out


_NC_CACHE = {}


def kernel(Q, K, V, Wq, bq, Wk, bk, Wv, bv, Wo, bo):
    if "nc" not in _NC_CACHE:
        _NC_CACHE["nc"] = build_nc()
    nc = _NC_CACHE["nc"]
    in_maps = shard_inputs(Q, K, V, Wq, bq, Wk, bk, Wv, bv, Wo, bo)
    res = run_bass_kernel_spmd(nc, in_maps, core_ids=list(range(N_CORES)))
    return unshard(res.results, bo)



# revision 12
# speedup vs baseline: 8.1518x; 8.1518x over previous
"""Multi-head attention (B=2, S=2048, D=1024, H=16) on 8 Trainium2 cores.

Sharding: core c -> (batch b = c//4, head-group hg = c%4 of 4 heads, d_h=256).
Megatron-style: column-shard W_{q,k,v}, row-shard W_o; partial outputs are
summed on the host (the unshard step).

Per-core pipeline (activations kept transposed, "T-space", fp32r matmuls):
  kT = (Wk_hg/8) @ K_b^T        [256, 2048]
  v  = V_b @ Wv_hg^T (+ones col)[2048, 4*65]
  per 512-wide q-chunk qc:
    qT[:, qc] = Wq_hg @ Q_b^T[:, qc]      (interleaved into attention as
                                           fillers to keep PE busy while
                                           the ScalarE runs exp)
    per head pair, per kt:
      sT[128,1024] = two row-tiled K=64 matmuls (hl0 -> cols :512, hl1 512:)
      pT = exp(sT)           (one ScalarE instr over 2 PSUM banks)
      ctxT (+denom row) = v_h_aug^T @ pT   (M=65, accumulated over 16 kt)
    normalize: DVE recip row -> PE outer-product broadcast (K=1 matmul)
               -> DVE copy -> DVE mult  (no DRAM roundtrip)
    oT_partial[:, qc] = Wo_cols^T-slices @ ctxT
Host: out[b] = (sum over the 4 cores of batch b of oT).T + bo.
"""

from collections import deque

import numpy as np

import bass_rust
import concourse.bass as bass
import concourse.mybir as mybir
import concourse.tile as tile
from concourse.bass_utils import run_bass_kernel_spmd

F32 = mybir.dt.float32
F32R = mybir.dt.float32r

B, S, D = 2, 2048, 1024
H = 16
DK = 64
N_CORES = 8
HEADS_PER_CORE = 4          # d_h = 256
DH = HEADS_PER_CORE * DK    # 256
VW = DK + 1                 # v columns per head incl. ones column
NV = HEADS_PER_CORE * VW    # 260
QC = 512                    # q-chunk (PSUM bank = 512 fp32)
N_QC = S // QC              # 4
N_KT = S // 128             # 16 key tiles
N_KO = D // 128             # 8 contraction tiles for projections
MT = DH // 128              # 2 m-tiles for qT/kT/ctxT


def _legalize_waits(nc):
    """walrus here allows 1 sync-wait per instruction (2 for EventSemaphore);
    Tile emits more. Spill extras onto same-engine NoOps placed just before."""
    caps = {"InstEventSemaphore": 2}
    n_nops = 0
    for f in nc.m.functions:
        for bb in f.blocks:
            insts = bb.instructions
            out = []
            changed = False
            for inst in insts:
                si = inst.sync_info
                waits = list(si.on_wait) if si is not None else []
                cap = caps.get(type(inst).__name__, 1)
                if len(waits) > cap:
                    spill, keep = waits[:-cap], waits[-cap:]
                    for w in spill:
                        nop = mybir.InstNoOp(name=f"waitfix-{n_nops}", ins=[], outs=[])
                        n_nops += 1
                        nop.engine = inst.engine
                        nop.sync_info = bass_rust.SyncInfo(on_wait=[w], on_update=[])
                        out.append(nop)
                    si.on_wait = keep
                    changed = True
                out.append(inst)
            if changed:
                insts[:] = out
    return n_nops


def build_nc(reps: int = 1):
    nc = bass.Bass(num_devices=N_CORES)

    t = {}
    t["xqT"] = nc.dram_tensor("xqT", [D, S], F32, kind="ExternalInput")
    t["xkT"] = nc.dram_tensor("xkT", [D, S], F32, kind="ExternalInput")
    t["xvT"] = nc.dram_tensor("xvT", [D, S], F32, kind="ExternalInput")
    t["wqT"] = nc.dram_tensor("wqT", [D, DH], F32, kind="ExternalInput")
    t["wkT"] = nc.dram_tensor("wkT", [D, DH], F32, kind="ExternalInput")
    t["wvT"] = nc.dram_tensor("wvT", [D, NV], F32, kind="ExternalInput")
    t["bq"] = nc.dram_tensor("bq", [DH], F32, kind="ExternalInput")
    t["bk"] = nc.dram_tensor("bk", [DH], F32, kind="ExternalInput")
    t["bv_bc"] = nc.dram_tensor("bv_bc", [128, NV], F32, kind="ExternalInput")
    t["ones_bc"] = nc.dram_tensor("ones_bc", [128, DK], F32, kind="ExternalInput")
    t["woT"] = nc.dram_tensor("woT", [DH, D], F32, kind="ExternalInput")
    t["oT"] = nc.dram_tensor("oT", [D, S], F32, kind="ExternalOutput")

    with tile.TileContext(nc) as tc:
        _body(nc, tc, t, reps)
    _legalize_waits(nc)
    return nc


def _body(nc, tc, t, reps):
    from contextlib import ExitStack

    with ExitStack() as ctx:
        singles = ctx.enter_context(tc.tile_pool(name="singles", bufs=1))

        wq_s = singles.tile([128, N_KO, DH], F32R)
        wk_s = singles.tile([128, N_KO, DH], F32R)
        wv_s = singles.tile([128, N_KO, NV], F32R)
        wo_s = singles.tile([128, MT, D], F32R)
        bq_s = singles.tile([128, MT], F32)
        bk_s = singles.tile([128, MT], F32)
        bv_s = singles.tile([128, NV], F32)
        ones_s = singles.tile([128, DK], F32R)
        # weights on the SWDGE (Pool) queue so the input-x loads own the
        # sync/HWDGE queue from t=0.
        nc.gpsimd.dma_start(wq_s[:], t["wqT"].rearrange("(ko p) m -> p ko m", p=128).bitcast(F32R))
        nc.gpsimd.dma_start(wk_s[:], t["wkT"].rearrange("(ko p) m -> p ko m", p=128).bitcast(F32R))
        nc.gpsimd.dma_start(wv_s[:], t["wvT"].rearrange("(ko p) m -> p ko m", p=128).bitcast(F32R))
        nc.gpsimd.dma_start(wo_s[:], t["woT"].rearrange("(kt p) e -> p kt e", p=128).bitcast(F32R))
        nc.gpsimd.dma_start(bq_s[:], t["bq"].rearrange("(m p) -> p m", p=128))
        nc.gpsimd.dma_start(bk_s[:], t["bk"].rearrange("(m p) -> p m", p=128))
        nc.gpsimd.dma_start(bv_s[:], t["bv_bc"][:, :])
        nc.gpsimd.dma_start(ones_s[:], t["ones_bc"][:, :].bitcast(F32R))

        qT_s = singles.tile([128, MT, S], F32R)
        ctxT_s = singles.tile([128, MT, S], F32R)

        kv_pools = {
            "kT": ctx.enter_context(tc.tile_pool(name="kT", bufs=2)),
            "v": ctx.enter_context(tc.tile_pool(name="v", bufs=2)),
        }
        xin = ctx.enter_context(tc.tile_pool(name="xin", bufs=2))
        pT_pool = ctx.enter_context(tc.tile_pool(name="pT", bufs=3))
        norm_pool = ctx.enter_context(tc.tile_pool(name="norm", bufs=2))
        osb_pool = ctx.enter_context(tc.tile_pool(name="osb", bufs=1))
        proj_ps = ctx.enter_context(tc.tile_pool(name="proj_ps", bufs=2, space="PSUM"))
        s_ps = ctx.enter_context(tc.tile_pool(name="s_ps", bufs=2, space="PSUM"))
        ctx_ps = ctx.enter_context(tc.tile_pool(name="ctx_ps", bufs=1, space="PSUM"))

        pools = dict(
            xin=xin, pT=pT_pool, norm=norm_pool, osb=osb_pool,
            proj_ps=proj_ps, s_ps=s_ps, ctx_ps=ctx_ps, **kv_pools,
        )
        consts = dict(
            wq_s=wq_s, wk_s=wk_s, wv_s=wv_s, wo_s=wo_s,
            bq_s=bq_s, bk_s=bk_s, bv_s=bv_s, ones_s=ones_s,
            qT_s=qT_s, ctxT_s=ctxT_s,
        )
        _compute_stream(nc, t, pools, consts, reps)


def _qk_proj_gen(nc, pools, t, xname, w_s, b_s, dst, sc):
    """Yield-per-piece projection of one 512-col chunk of qT or kT."""
    xt = pools["xin"].tile([128, N_KO, QC], F32R, tag="x", name="xt")
    nc.sync.dma_start(
        xt[:],
        t[xname].rearrange("(ko p) s -> p ko s", p=128)[
            :, :, sc * QC:(sc + 1) * QC
        ].bitcast(F32R),
    )
    yield
    for m in range(MT):
        ps = pools["proj_ps"].tile([128, QC], F32, tag="proj", name="proj_ps")
        for ko in range(0, N_KO, 2):
            nc.tensor.matmul(
                ps[:], w_s[:, ko, m * 128:(m + 1) * 128], xt[:, ko],
                start=(ko == 0), stop=False,
            )
            nc.tensor.matmul(
                ps[:], w_s[:, ko + 1, m * 128:(m + 1) * 128], xt[:, ko + 1],
                start=False, stop=(ko + 1 == N_KO - 1),
            )
            yield
        nc.vector.tensor_tensor(
            dst[:, m, sc * QC:(sc + 1) * QC],
            ps[:],
            b_s[:, m, None].to_broadcast((128, QC)),
            mybir.AluOpType.add,
        )
        yield


def _v_proj_gen(nc, pools, t, wv_s, bv_s, v_t, sc):
    xt = pools["xin"].tile([128, N_KO, QC], F32R, tag="x", name="xt")
    nc.sync.dma_start(
        xt[:],
        t["xvT"].rearrange("(ko p) s -> p ko s", p=128)[
            :, :, sc * QC:(sc + 1) * QC
        ].bitcast(F32R),
    )
    yield
    for rt in range(QC // 128):
        ps = pools["proj_ps"].tile([128, QC], F32, tag="proj", name="proj_ps")
        for ko in range(0, N_KO, 2):
            nc.tensor.matmul(
                ps[:, :NV], xt[:, ko, rt * 128:(rt + 1) * 128], wv_s[:, ko],
                start=(ko == 0), stop=False,
            )
            nc.tensor.matmul(
                ps[:, :NV], xt[:, ko + 1, rt * 128:(rt + 1) * 128], wv_s[:, ko + 1],
                start=False, stop=(ko + 1 == N_KO - 1),
            )
            yield
        nc.vector.tensor_tensor(
            v_t[:, sc * (QC // 128) + rt, :], ps[:, :NV], bv_s[:],
            mybir.AluOpType.add,
        )
        yield


def _norm_stages(nc, pools, c, ctx_banks, r_tiles, m, qc):
    """Softmax normalization tail for one head pair (recips already emitted):
    stage 1 = PE outer-product broadcast of the reciprocals, stage 2 = DVE
    copy + scale into ctxT. Stepped at kt==0 / kt==1 of the NEXT pair so the
    emission (and thus dependency) order stays: final PV -> recip -> bc ->
    mult -> next pair's PV (which reuses the ctx banks)."""
    ctxT_s = c["ctxT_s"]
    bc = pools["s_ps"].tile([128, 2 * QC], F32, tag="s", name="bc")
    for hl in range(2):
        nc.tensor.matmul(
            bc[0:DK, hl * QC:(hl + 1) * QC],
            c["ones_s"][64:65, :],
            r_tiles[hl][64:65, :],
            start=True,
            stop=True,
        )
    yield
    for hl in range(2):
        rbc = pools["norm"].tile([128, QC], F32, tag="rbc", name="rbc")
        nc.vector.tensor_copy(rbc[0:DK, :], bc[0:DK, hl * QC:(hl + 1) * QC])
        nc.vector.tensor_tensor(
            ctxT_s[64 * hl:64 * hl + 64, m, qc * QC:(qc + 1) * QC],
            ctx_banks[hl][0:DK, :],
            rbc[0:DK, :],
            mybir.AluOpType.mult,
        )
    yield


def _oproj_gen(nc, pools, t, c, qc):
    """Output projection for one q-chunk; runs as filler after _norm_gen."""
    ctxT_s = c["ctxT_s"]
    o_sb = pools["osb"].tile([128, D // 128, QC], F32, tag="o", name="o_sb")
    for mt in range(D // 128):
        ps_o = pools["proj_ps"].tile([128, QC], F32, tag="proj", name="ps_o")
        for kt in range(MT):
            nc.tensor.matmul(
                ps_o[:],
                c["wo_s"][:, kt, mt * 128:(mt + 1) * 128],
                ctxT_s[:, kt, qc * QC:(qc + 1) * QC],
                start=(kt == 0),
                stop=(kt == MT - 1),
            )
        nc.vector.tensor_copy(o_sb[:, mt, :], ps_o[:])
        yield
    nc.gpsimd.dma_start(
        t["oT"].rearrange("(mt p) s -> p mt s", p=128)[
            :, :, qc * QC:(qc + 1) * QC
        ],
        o_sb[:],
    )
    yield


PV_DELAY = 2  # kt lag between score/exp emission and the consuming PV


def _compute_stream(nc, t, pools, c, reps):
    """Single fully-pipelined emission stream over reps x q-chunks x pairs.

    Deferred work (output projection, this rep's next Q-chunk projection,
    the NEXT rep's K/V/Q0 projections) lives in a FIFO of generators pumped
    piecewise inside the attention kt loops, so the PE always has runnable
    instructions while the ScalarE works through the exps. Normalization of
    pair u is emitted inline at the start of pair u+1's kt loop (before the
    first PV of u+1, which reuses the ctx PSUM banks)."""
    from collections import deque

    qT_s = c["qT_s"]

    def kv_alloc():
        return (
            pools["kT"].tile([128, MT, S], F32R, tag="kT", name="kT_t"),
            pools["v"].tile([128, N_KT, NV], F32R, tag="v", name="v_t"),
        )

    def prologue_gens(kv):
        kT_t, v_t = kv
        gens = [
            _qk_proj_gen(nc, pools, t, "xkT", c["wk_s"], c["bk_s"], kT_t, sc)
            for sc in range(N_QC)
        ]
        gens.append(_qk_proj_gen(nc, pools, t, "xqT", c["wq_s"], c["bq_s"], qT_s, 0))
        gens.extend(
            _v_proj_gen(nc, pools, t, c["wv_s"], c["bv_s"], v_t, sc)
            for sc in range(N_QC)
        )
        return gens

    fill = deque()

    def pump(n):
        done = 0
        while done < n and fill:
            try:
                next(fill[0])
                done += 1
            except StopIteration:
                fill.popleft()

    def force(gens):
        """Run specific generators to completion (correctness barrier)."""
        for g in gens:
            for _ in g:
                pass

    # rep 0 prologue runs inline (cold start).
    kv = kv_alloc()
    force(prologue_gens(kv))
    kv_next = None
    staged = []          # next-rep prologue gens not yet queued
    queued_pro = []      # next-rep prologue gens already in fill
    qproj_gen = None     # this rep's next Q-chunk projection

    # unit = (rep, qc, pair) in stream order
    units = [
        (rep, qc, pair)
        for rep in range(reps)
        for qc in range(N_QC)
        for pair in range(HEADS_PER_CORE // 2)
    ]

    norm_pending = None   # (stage_gen, qc_of_norm, pair_of_norm)
    oproj_wait = []       # qc indices whose oproj awaits norm completion

    for rep, qc, pair in units:
        if qc == 0 and pair == 0:
            # rep boundary: prologue for this rep must be fully emitted.
            if rep > 0:
                force(queued_pro + staged)
                queued_pro, staged = [], []
                kv = kv_next
            kT_t, v_t = kv
        if pair == 0:
            # stage next-rep prologue into the filler queue
            if rep + 1 < reps:
                if qc == 1:
                    kv_next = kv_alloc()
                    staged = prologue_gens(kv_next)
                    take, staged = staged[:3], staged[3:]
                    fill.extend(take)
                    queued_pro += take
                elif qc in (2, 3):
                    take, staged = staged[:3], staged[3:]
                    fill.extend(take)
                    queued_pro += take
            # this rep's next q-chunk projection
            if qc > 0 and qproj_gen is not None:
                force([qproj_gen])  # must be done before this qc's scores
            if qc + 1 < N_QC:
                qproj_gen = _qk_proj_gen(
                    nc, pools, t, "xqT", c["wq_s"], c["bq_s"], qT_s, qc + 1
                )
                fill.append(qproj_gen)

        m = pair
        ctx_banks = [
            pools["ctx_ps"].tile([128, QC], F32, tag=f"ctx{hl}", name=f"ctx{hl}")
            for hl in range(2)
        ]
        pend = deque()
        for kt in range(N_KT + PV_DELAY):
            if kt < N_KT:
                s2 = pools["s_ps"].tile([128, 2 * QC], F32, tag="s", name="s2")
                for hl in range(2):
                    off = 64 * hl
                    nc.tensor.matmul(
                        s2[:, hl * QC:(hl + 1) * QC],
                        kT_t[off:off + 64, m, kt * 128:(kt + 1) * 128],
                        qT_s[off:off + 64, m, qc * QC:(qc + 1) * QC],
                        start=True,
                        stop=True,
                    )
                pt = pools["pT"].tile([128, 2 * QC], F32R, tag="p", name="pt")
                nc.scalar.activation(
                    pt[:], s2[:], mybir.ActivationFunctionType.Exp
                )
                pend.append((pt, kt))
            # previous pair's normalization, interleaved ahead of our PVs
            if norm_pending is not None and kt <= 1:
                ng, nqc, npair = norm_pending
                next(ng, None)
                if kt == 1:
                    next(ng, None)
                    norm_pending = None
                    if npair == 1:
                        fill.append(_oproj_gen(nc, pools, t, c, nqc))
            if kt >= PV_DELAY:
                ppt, pkt = pend.popleft()
                for hl in range(2):
                    h = 2 * pair + hl
                    nc.tensor.matmul(
                        ctx_banks[hl][0:VW, :],
                        v_t[:, pkt, h * VW:(h + 1) * VW],
                        ppt[:, hl * QC:(hl + 1) * QC],
                        start=(pkt == 0),
                        stop=(pkt == N_KT - 1),
                    )
            pump(2)
        # reciprocals for this pair now; bc/mult stages run in the next unit
        r_tiles = []
        for hl in range(2):
            r_s = pools["norm"].tile([128, QC], F32R, tag="r", name="r_s")
            with nc.allow_low_precision(reason="recip feeds f32r broadcast mm"):
                nc.vector.reciprocal(r_s[64:65, :], ctx_banks[hl][64:65, :])
            r_tiles.append(r_s)
        norm_pending = (
            _norm_stages(nc, pools, c, ctx_banks, r_tiles, m, qc),
            qc,
            pair,
        )

    # stream tail: final normalization + last oproj + leftovers
    ng, nqc, npair = norm_pending
    for _ in ng:
        pass
    fill.append(_oproj_gen(nc, pools, t, c, nqc))
    while fill:
        pump(100)


def shard_inputs(Q, K, V, Wq, bq, Wk, bk, Wv, bv, Wo, bo):
    """Host-side shard prep. Returns per-core in_maps."""
    scale = 1.0 / np.sqrt(np.float32(DK))
    in_maps = []
    xT = {}
    for b in range(B):
        xT[b] = (
            np.ascontiguousarray(np.asarray(Q[b]).T),
            np.ascontiguousarray(np.asarray(K[b]).T),
            np.ascontiguousarray(np.asarray(V[b]).T),
        )
    for c in range(N_CORES):
        b, hg = c // HEADS_PER_CORE, c % HEADS_PER_CORE
        rows = slice(DH * hg, DH * (hg + 1))
        wqT = np.ascontiguousarray(np.asarray(Wq)[rows].T)
        wkT = np.ascontiguousarray((np.asarray(Wk)[rows] * scale).T)
        wvT = np.zeros((D, NV), np.float32)
        bv_bc = np.zeros((128, NV), np.float32)
        for i in range(HEADS_PER_CORE):
            wr = slice(DH * hg + DK * i, DH * hg + DK * (i + 1))
            wvT[:, VW * i:VW * i + DK] = np.asarray(Wv)[wr].T
            bv_bc[:, VW * i:VW * i + DK] = np.asarray(bv)[wr][None, :]
            bv_bc[:, VW * i + DK] = 1.0
        woT = np.ascontiguousarray(np.asarray(Wo)[:, rows].T)
        in_maps.append(
            {
                "xqT": xT[b][0],
                "xkT": xT[b][1],
                "xvT": xT[b][2],
                "wqT": wqT,
                "wkT": wkT,
                "wvT": wvT,
                "bq": np.ascontiguousarray(np.asarray(bq)[rows]),
                "bk": np.ascontiguousarray(np.asarray(bk)[rows] * scale),
                "bv_bc": bv_bc,
                "ones_bc": np.ones((128, DK), np.float32),
                "woT": woT,
            }
        )
    return in_maps


def unshard(results, bo):
    out = np.empty((B, S, D), np.float32)
    for b in range(B):
        acc = results[b * HEADS_PER_CORE]["oT"].astype(np.float32).copy()
        for hg in range(1, HEADS_PER_CORE):
            acc += results[b * HEADS_PER_CORE + hg]["oT"]
        out[b] = acc.T + np.asarray(bo)[None, :]
    return out


_NC_CACHE = {}


def kernel(Q, K, V, Wq, bq, Wk, bk, Wv, bv, Wo, bo):
    if "nc" not in _NC_CACHE:
        _NC_CACHE["nc"] = build_nc()
    nc = _NC_CACHE["nc"]
    in_maps = shard_inputs(Q, K, V, Wq, bq, Wk, bk, Wv, bv, Wo, bo)
    res = run_bass_kernel_spmd(nc, in_maps, core_ids=list(range(N_CORES)))
    return unshard(res.results, bo)
